# revision 6
# baseline (speedup 1.0000x reference)
"""Trainium2 Bass kernel for DifferentiableDLT (batched weighted-DLT homography fit).

Contract: kernel(**inputs) takes FULL inputs
    flow (64, 2, 320, 576) f32, mask (64, 1, 320, 576) f32, img_h, img_w
and returns the FULL output (64, 3, 3) f32.

v2 design (pure data parallel, 8 batches/core x 8 cores):
  The 1024 sample points form a fixed separable 32x32 grid; bilinear sampling
  touches 64 rows (32 pairs) x 64 cols.  y0(k) = 16 + 37*kd + 9*s is affine in
  (kd = k//4, s = k%4) except k=31 (+1), so the needed row-pairs are fetched
  with plain 3-dim strided HWDGE DMAs (15 of them incl. k=31 fixups) instead
  of SWDGE gathers — transfers start ~1 us after kernel main.
  Per core:
    1. Row-pair DMAs into tF [128 p=(c,kd,b), s, 1152] and tM [64 p=(kd,b), s,
       (a,w)]; all weights/matrices arrive in 3 packed constant DMAs.
    2. gpsimd ap_gather selects the 64 needed columns (4 taps per point) in
       one instruction per data half; one DVE multiply by a fused 4-tap
       bilinear weight tile (image-scale folded in) + two tree-adds gives the
       scaled samples.
    3. PE transpose (identity matmul) + grid-offset matmul -> dst image
       coords in PSUM, points on partitions.
    4. Hartley stats via row-sum + ones-matmul broadcast; radius via
       sqrt + ones-matmul; weighted feature products D = [w, w*cx, w*cy,
       w*r2]; moments C^T @ D on the PE.
    5. Normal equations assembled by a PE matmul against EQG = M0inv @ E,
       which directly yields the Jacobi-preconditioned system G = I - M0inv*A
       and c0 = M0inv*b (M0 = host-constant ideal normal matrix).  Solved by
       6 Horner steps of the Neumann series (spectral radius ~0.09).
    6. Denormalize H, sign/scale fix, support gate, DMA out (8,3,3).
"""

import dataclasses
import math
import numpy as np

import concourse.bass as bass
import concourse.bacc as bacc
import concourse.mybir as mybir
from concourse import tile
from concourse import bass_utils

F32 = mybir.dt.float32
I16 = mybir.dt.int16
ALU = mybir.AluOpType
ACTF = mybir.ActivationFunctionType

NCORES = 8
BPC = 8          # batches per core
HF, WF = 320, 576
HW = HF * WF
NG = 32          # grid is NG x NG points
NPTS = NG * NG
EPS = 1e-6
KHORNER = 6      # Neumann/Horner applications

# ---------------------------------------------------------------------------
# host-side constant computation
# ---------------------------------------------------------------------------


def _grid_1d(size, n):
    m = int(size * 0.05)
    return np.linspace(m, size - m - 1, n, dtype=np.float32)


class _Consts:
    def __init__(self, img_h, img_w):
        ys = _grid_1d(HF, NG)
        xs = _grid_1d(WF, NG)
        y0 = np.floor(ys).astype(np.int64)
        x0 = np.floor(xs).astype(np.int64)
        wy = (ys - y0).astype(np.float64)
        wx = (xs - x0).astype(np.float64)
        sx = np.float64(np.float32((img_w - 1) / max(WF - 1, 1)))
        sy = np.float64(np.float32((img_h - 1) / max(HF - 1, 1)))

        # ---- 4-tap interp weights ----
        # W4F [128, (s,i,a,c2)=512], p = c*64 + kd*8 + b
        p = np.arange(128)
        kd_f = (p % 64) // 8
        c_f = p // 64
        wya = np.stack([1 - wy, wy], -1)        # (32, 2) [k, a]
        wxc = np.stack([1 - wx, wx], -1)        # (32, 2) [i, c2]
        s_ = np.arange(4)
        k_ps = kd_f[:, None] * 4 + s_[None, :]  # (128, 4)
        sxy_f = np.where(c_f == 0, sx, sy)
        W4F = (wya[k_ps][:, :, None, :, None] * wxc[None, None, :, None, :]
               * sxy_f[:, None, None, None, None]).reshape(128, 512)
        self.W4F = W4F.astype(np.float32)
        pm = np.arange(64)
        k_pm = (pm // 8)[:, None] * 4 + s_[None, :]
        W4M = (wya[k_pm][:, :, None, :, None] * wxc[None, None, :, None, :]
               ).reshape(64, 512)
        self.W4M = W4M.astype(np.float32)

        # ---- x-select ap_gather index table (shared flow/mask) ----
        xidx = np.zeros(256, np.int64)
        for s2 in range(2):
            for i in range(NG):
                for a in range(2):
                    for c2 in range(2):
                        pos = ((s2 * NG + i) * 2 + a) * 2 + c2
                        xidx[pos] = s2 * 1152 + a * 576 + x0[i] + c2
        base = np.zeros((16, 16), np.int16)
        for k, vv in enumerate(xidx):
            base[k % 16, k // 16] = vv
        self.XIDX = np.tile(base, (8, 1))

        # ---- grid-offset matmul constants, j = (c, kd, b) ----
        jj = np.arange(128)
        c_j = jj // 64
        kd_j = (jj % 64) // 8
        ff = np.arange(128)
        G5 = np.zeros((5, 128))
        GR5 = np.zeros((5, 128))
        for sp in range(4):
            G5[sp] = (ff // 32 == sp)
            GR5[sp] = np.where(c_j == 1, ys.astype(np.float64)[4 * kd_j + sp] * sy, 0.0)
        G5[4] = xs.astype(np.float64)[ff % 32]
        GR5[4] = np.where(c_j == 0, sx, 0.0)
        self.G5 = G5.astype(np.float32)
        self.GR5 = GR5.astype(np.float32)

        # ---- means route: BM = CBMASK * rowsum + GMC; MB = ones^T @ BM ----
        j2 = np.arange(16)
        c_j2 = j2 // 8
        b_j2 = j2 % 8
        self.CBMASK = (((c_f[:, None] == c_j2[None, :])
                        & ((p % 8)[:, None] == b_j2[None, :]))
                       .astype(np.float32) / NPTS)
        gmean = np.where(c_j2 == 0, xs.astype(np.float64).mean() * sx,
                         ys.astype(np.float64).mean() * sy)
        self.GMC = (np.ones((128, 1)) * gmean[None, :] / 128.0).astype(np.float32)

        # ---- source-point features + T_src immediates ----
        jpt = np.arange(NPTS) // NG
        ipt = np.arange(NPTS) % NG
        gx = xs.astype(np.float64)[ipt]
        gy = ys.astype(np.float64)[jpt]
        sxi = gx * sx
        syi = gy * sy
        mx0, my0 = sxi.mean(), syi.mean()
        cxs, cys = sxi - mx0, syi - my0
        s_src = max(np.sqrt(cxs * cxs + cys * cys).mean() / math.sqrt(2.0), 1e-8)
        u = cxs / s_src
        v = cys / s_src
        self.a_ts = float(np.float32(1.0 / s_src))
        self.c_ts = float(np.float32(-mx0 / s_src))
        self.d_ts = float(np.float32(-my0 / s_src))
        feats = np.stack([u * u, u * v, u, v * v, v, np.ones_like(u)], -1)
        self.C6 = np.ascontiguousarray(
            feats.reshape(8, 128, 6).transpose(1, 0, 2).reshape(128, 48)
        ).astype(np.float32)

        # ---- E matrices: AUG[r*9+c] = sum_q sum_m E[q][m, r*9+c] * Mq[m] ----
        E = np.zeros((4, 6, 72))
        sym = [[0, 1, 2], [1, 3, 4], [2, 4, 5]]
        for r in range(3):
            for c in range(3):
                m = sym[r][c]
                E[0, m, r * 9 + c] += 1
                E[0, m, (r + 3) * 9 + (c + 3)] += 1
        cr = [[0, 1], [1, 3], [2, 4]]
        for q, r0 in ((1, 0), (2, 3)):
            for r in range(3):
                for c2 in range(2):
                    m = cr[r][c2]
                    E[q, m, (r0 + r) * 9 + 6 + c2] += -1
                    E[q, m, (6 + c2) * 9 + (r0 + r)] += -1
            for r, m in ((0, 2), (1, 4), (2, 5)):
                E[q, m, (r0 + r) * 9 + 8] += 1
        rb = [[0, 1], [1, 3]]
        for r in range(2):
            for c2 in range(2):
                E[3, rb[r][c2], (6 + r) * 9 + 6 + c2] += 1
        E[3, 2, 6 * 9 + 8] += -1
        E[3, 4, 7 * 9 + 8] += -1

        # ---- M0 (ideal normal matrix) -> EQG = M0inv @ E, IME = I-eps*M0inv
        o = np.ones_like(u)
        z = np.zeros_like(u)
        r1 = np.stack([u, v, o, z, z, z, -u * u, -u * v], -1)
        r2 = np.stack([z, z, z, u, v, o, -v * u, -v * v], -1)
        A0 = np.concatenate([r1, r2], 0) * math.sqrt(0.5)
        M0 = A0.T @ A0 + EPS * np.eye(8)
        M0inv = np.linalg.inv(M0)
        EQG = np.einsum('ir,qmrc->qmic', M0inv,
                        E.reshape(4, 6, 8, 9)).reshape(4, 6, 72)
        EQG73 = np.zeros((4, 6, 73))
        EQG73[:, :, 0:72] = EQG
        EQG73[0, 5, 72] = 1.0  # col 72 of q=0 block picks S1 = sum(w)
        self.EQG = np.ascontiguousarray(
            EQG73.transpose(1, 0, 2).reshape(6, 292)).astype(np.float32)
        self.IME = np.tile((np.eye(8) - EPS * M0inv).reshape(1, 64),
                           (8, 1)).astype(np.float32)

        # ---- packed constant blobs ----
        IDN = np.eye(128, dtype=np.float32)
        cm = np.zeros((128, 720), np.float32)
        cm[:, 0:512] = self.W4F
        cm[:, 512:640] = IDN
        cm[:, 640:688] = self.C6
        cm[:, 688:704] = self.CBMASK
        cm[:, 704:720] = self.GMC
        self.CMAIN = cm
        self.CW4M = self.W4M
        cs = np.zeros((8, 612), np.float32)
        cs[0:5, 0:128] = self.G5
        cs[0:5, 128:256] = self.GR5
        cs[0:6, 256:548] = self.EQG
        cs[0:8, 548:612] = self.IME
        self.CSMALL = cs


# ---------------------------------------------------------------------------
# device program
# ---------------------------------------------------------------------------


def _build_program(cc: _Consts):
    nc = bacc.Bacc("TRN2", target_bir_lowering=False, debug=False,
                   num_swdge_queues=1)

    fm = nc.dram_tensor("fm", [BPC, 3, HF, WF], F32, kind="ExternalInput")
    CMAIN = nc.dram_tensor("CMAIN", [128, 720], F32, kind="ExternalInput")
    CW4M = nc.dram_tensor("CW4M", [64, 512], F32, kind="ExternalInput")
    CSMALL = nc.dram_tensor("CSMALL", [8, 612], F32, kind="ExternalInput")
    XIDX = nc.dram_tensor("XIDX", [128, 16], I16, kind="ExternalInput")
    Hout = nc.dram_tensor("H", [BPC, 3, 3], F32, kind="ExternalOutput")

    V = nc.vector
    A = nc.scalar
    T = nc.tensor
    G = nc.gpsimd
    S = nc.sync

    fmflat = fm.ap().rearrange("b c h w -> (b c h w)").unsqueeze(0)

    def fsrc(base, ap):
        return dataclasses.replace(fmflat, ap=ap, offset=base)

    with tile.TileContext(nc) as tc:
        with (
            tc.tile_pool(name="sb", bufs=1) as pool,
            tc.tile_pool(name="ps", bufs=1, space="PSUM") as psp,
        ):
            # ---------------- constant loads (A queue) ----------------
            CMAIN_t = pool.tile([128, 720], F32, tag="CMAIN")
            A.dma_start(CMAIN_t[:, :], CMAIN.ap())
            CW4M_t = pool.tile([64, 512], F32, tag="CW4M")
            A.dma_start(CW4M_t[:, :], CW4M.ap())
            CSMALL_t = pool.tile([8, 612], F32, tag="CSMALL")
            A.dma_start(CSMALL_t[:, :], CSMALL.ap())
            XIDX_t = pool.tile([128, 16], I16, tag="XIDX")
            A.dma_start(XIDX_t[:, :], XIDX.ap())

            W4F_t = CMAIN_t[:, 0:512]
            IDN_t = CMAIN_t[:, 512:640]
            C6_t = CMAIN_t[:, 640:688]
            CBMASK_t = CMAIN_t[:, 688:704]
            GMC_t = CMAIN_t[:, 704:720]
            G5_t = CSMALL_t[0:5, 0:128]
            GR5_t = CSMALL_t[0:5, 128:256]
            EQG_t = CSMALL_t[0:6, 256:548]
            IME_t = CSMALL_t[0:8, 548:612]

            # ---------------- row-pair DMAs (SP queue) ----------------
            tF = pool.tile([128, 4, 1152], F32)   # p=(c,kd,b)
            tM = pool.tile([64, 4, 1152], F32)    # p=(kd,b), free (s, a*576+w)
            for s in (0, 1, 2):
                for c in (0, 1):
                    S.dma_start(
                        tF[64 * c:64 * c + 64, s, :],
                        fsrc(c * HW + (16 + 9 * s) * WF,
                             [[37 * WF, 8], [3 * HW, 8], [1, 1152]]))
            for c in (0, 1):
                S.dma_start(
                    tF[64 * c:64 * c + 56, 3, :],
                    fsrc(c * HW + 43 * WF, [[37 * WF, 7], [3 * HW, 8], [1, 1152]]))
                S.dma_start(
                    tF[64 * c + 56:64 * c + 64, 3, :],
                    fsrc(c * HW + 303 * WF, [[3 * HW, 8], [1, 1152]]))
            for s in (0, 1, 2):
                S.dma_start(
                    tM[0:64, s, :],
                    fsrc(2 * HW + (16 + 9 * s) * WF,
                         [[37 * WF, 8], [3 * HW, 8], [1, 1152]]))
            S.dma_start(
                tM[0:56, 3, :],
                fsrc(2 * HW + 43 * WF, [[37 * WF, 7], [3 * HW, 8], [1, 1152]]))
            S.dma_start(
                tM[56:64, 3, :],
                fsrc(2 * HW + 303 * WF, [[3 * HW, 8], [1, 1152]]))

            # ---------------- memsets + ACT warmup ----------------
            ONESB = pool.tile([128, 128], F32)
            V.memset(ONESB[:, :], 1.0)
            ONESC = pool.tile([128, 1], F32)
            V.memset(ONESC[:, :], 1.0 / NPTS)
            ONES6R = pool.tile([1, 6], F32)
            V.memset(ONES6R[:, :], 1.0)
            IEYE = pool.tile([8, 9], F32)
            V.memset(IEYE[:, :], 0.0)
            V.memset(IEYE[:, 0:9:4], 1.0)
            HN = pool.tile([8, 9], F32)
            V.memset(HN[:, 8:9], 1.0)
            ACTJ = pool.tile([8, 2], F32)
            V.memset(ACTJ[:, :], 1.0)
            A.activation(ACTJ[:, 0:1], ACTJ[:, 1:2], ACTF.Sqrt)
            A.activation(ACTJ[:, 1:2], ACTJ[:, 0:1], ACTF.Abs)
            A.activation(ACTJ[:, 0:1], ACTJ[:, 1:2], ACTF.Relu)

            # ---------------- flow halves: select + interp + transpose -----
            psF = psp.tile([128, 128], F32)
            samp = [None, None]
            rh = [None, None]
            for h in range(2):
                Gx = pool.tile([128, 256], F32, tag=f"Gx{h}")
                G.ap_gather(
                    out_ap=Gx[:, :],
                    in_ap=tF[:, 2 * h:2 * h + 2, :].rearrange("p s w -> p (s w)"),
                    idxs_ap=XIDX_t[:, :],
                    channels=128, num_elems=2304, d=1, num_idxs=256)
                P = pool.tile([128, 256], F32, tag=f"P{h}")
                V.tensor_tensor(out=P[:, :], in0=Gx[:, :],
                                in1=W4F_t[:, 256 * h:256 * h + 256], op=ALU.mult)
                Pv = P[:, :].rearrange("p (s i a c) -> p s i a c", s=2, i=32, a=2, c=2)
                Q = pool.tile([128, 128], F32, tag=f"Q{h}")
                Qv = Q[:, :].rearrange("p (s i a) -> p s i a", s=2, i=32, a=2)
                V.tensor_tensor(out=Qv, in0=Pv[:, :, :, :, 0], in1=Pv[:, :, :, :, 1],
                                op=ALU.add)
                sh = pool.tile([128, 64], F32, tag=f"sh{h}")
                shv = sh[:, :].rearrange("p (s i) -> p s i", s=2, i=32)
                V.tensor_tensor(out=shv, in0=Qv[:, :, :, 0], in1=Qv[:, :, :, 1],
                                op=ALU.add)
                samp[h] = sh
                T.matmul(psF[64 * h:64 * h + 64, :], sh[:, :], IDN_t,
                         start=True, stop=False)
                T.matmul(psF[64 * h:64 * h + 64, :], G5_t[:, 64 * h:64 * h + 64],
                         GR5_t, start=False, stop=True)
                rt = pool.tile([128, 1], F32, tag=f"rt{h}")
                V.tensor_reduce(out=rt[:, :], in_=sh[:, :].unsqueeze(1),
                                axis=mybir.AxisListType.X, op=ALU.add)
                rh[h] = rt

            # ---------------- means (r-route) ----------------
            RSUM = pool.tile([128, 1], F32)
            V.tensor_tensor(out=RSUM[:, :], in0=rh[0][:, :], in1=rh[1][:, :],
                            op=ALU.add)
            BM = pool.tile([128, 16], F32)
            V.scalar_tensor_tensor(out=BM[:, :], in0=CBMASK_t, scalar=RSUM[:, :],
                                   in1=GMC_t, op0=ALU.mult, op1=ALU.add)
            psMB = psp.tile([128, 16], F32)
            T.matmul(psMB[:, :], ONESB[:, :], BM[:, :], start=True, stop=True)
            MB = pool.tile([128, 16], F32)
            A.activation(MB[:, :], psMB[:, :], ACTF.Copy)

            # ---------------- centered coords + radius ----------------
            CXY = pool.tile([128, 128], F32)   # [pl, (c, t, b)]
            mbv = MB[:, :].rearrange("p (c b) -> p c b", c=2, b=8).unsqueeze(2)
            V.tensor_tensor(
                out=CXY[:, :].rearrange("p (c t b) -> p c t b", c=2, t=8, b=8),
                in0=psF[:, :].rearrange("p (c t b) -> p c t b", c=2, t=8, b=8),
                in1=mbv.broadcast_to([128, 2, 8, 8]), op=ALU.subtract)
            SQ = pool.tile([128, 128], F32, tag=f"Q{h}")
            V.tensor_tensor(out=SQ[:, :], in0=CXY[:, :], in1=CXY[:, :], op=ALU.mult)
            R2 = pool.tile([128, 64], F32)     # [pl, (t, b)]
            V.tensor_tensor(out=R2[:, :], in0=SQ[:, 0:64], in1=SQ[:, 64:128],
                            op=ALU.add)
            SQR = pool.tile([128, 64], F32)
            A.activation(SQR[:, :], R2[:, :], ACTF.Sqrt)
            psSq = psp.tile([1, 64], F32)
            T.matmul(psSq[:, :], ONESC[:, :], SQR[:, :], start=True, stop=True)
            sRow = pool.tile([1, 8], F32)
            V.tensor_reduce(
                out=sRow[:, :],
                in_=psSq[:, :].rearrange("o (t b) -> o b t", t=8, b=8),
                axis=mybir.AxisListType.X, op=ALU.add)
            V.tensor_scalar(out=sRow[:, :], in0=sRow[:, :],
                            scalar1=1.0 / math.sqrt(2.0), op0=ALU.mult,
                            scalar2=1e-8, op1=ALU.max)
            IR24 = pool.tile([1, 24], F32)
            V.reciprocal(IR24[:, 0:8], sRow[:, :])
            V.tensor_copy(IR24[:, 8:16], IR24[:, 0:8])
            V.tensor_tensor(out=IR24[:, 16:24], in0=IR24[:, 0:8], in1=IR24[:, 8:16],
                            op=ALU.mult)
            psC6 = psp.tile([6, 24], F32)
            T.matmul(psC6[:, :], ONES6R[:, :], IR24[:, :], start=True, stop=True)

            # ---------------- mask: select + interp + transpose -----------
            psM = psp.tile([128, 64], F32)
            D = pool.tile([128, 256], F32)
            Dv = D[:, :].rearrange("p (t q b) -> p t q b", t=8, q=4, b=8)
            sampM = pool.tile([64, 128], F32)
            for m in range(2):
                GxM = pool.tile([64, 256], F32, tag=f"GxM{m}")
                G.ap_gather(
                    out_ap=GxM[:, :],
                    in_ap=tM[0:64, 2 * m:2 * m + 2, :].rearrange("p s w -> p (s w)"),
                    idxs_ap=XIDX_t[0:64, :],
                    channels=64, num_elems=2304, d=1, num_idxs=256)
                PM = pool.tile([64, 256], F32, tag=f"PM{m}")
                V.tensor_tensor(out=PM[:, :], in0=GxM[:, :],
                                in1=CW4M_t[:, 256 * m:256 * m + 256], op=ALU.mult)
                PMv = PM[:, :].rearrange("p (s i a c) -> p s i a c",
                                         s=2, i=32, a=2, c=2)
                QM = pool.tile([64, 128], F32, tag=f"QM{m}")
                QMv = QM[:, :].rearrange("p (s i a) -> p s i a", s=2, i=32, a=2)
                V.tensor_tensor(out=QMv, in0=PMv[:, :, :, :, 0],
                                in1=PMv[:, :, :, :, 1], op=ALU.add)
                smv = sampM[:, 64 * m:64 * m + 64].rearrange(
                    "p (s i) -> p s i", s=2, i=32)
                V.tensor_tensor(out=smv, in0=QMv[:, :, :, 0], in1=QMv[:, :, :, 1],
                                op=ALU.add)
                T.matmul(psM[64 * m:64 * m + 64, :], sampM[:, 64 * m:64 * m + 64],
                         IDN_t[0:64, 0:64], start=True, stop=True)
                A.activation(Dv[64 * m:64 * m + 64, :, 0, :],
                             psM[64 * m:64 * m + 64, :], ACTF.Relu)

            # ---------------- D features + moments ----------------
            V.tensor_tensor(
                out=Dv[:, :, 1:3, :],
                in0=CXY[:, :].rearrange("p (c t b) -> p t c b", c=2, t=8, b=8),
                in1=Dv[:, :, 0:1, :].broadcast_to([128, 8, 2, 8]), op=ALU.mult)
            V.tensor_tensor(
                out=Dv[:, :, 3, :],
                in0=R2[:, :].rearrange("p (t b) -> p t b", t=8, b=8),
                in1=Dv[:, :, 0, :], op=ALU.mult)
            psMom = psp.tile([6, 32], F32)
            for t in range(8):
                T.matmul(psMom[:, :], C6_t[:, 6 * t:6 * t + 6],
                         D[:, 32 * t:32 * t + 32], start=(t == 0), stop=(t == 7))
            Msb = pool.tile([6, 32], F32)
            A.activation(Msb[:, :], psMom[:, :], ACTF.Copy)
            V.tensor_tensor(out=Msb[:, 8:32], in0=Msb[:, 8:32], in1=psC6[:, :],
                            op=ALU.mult)

            # ---------------- preconditioned normal equations ----------------
            psA = psp.tile([8, 73], F32)
            for q in range(4):
                T.matmul(psA[:, :], Msb[0:6, 8 * q:8 * q + 8],
                         EQG_t[:, 73 * q:73 * q + 73], start=(q == 0), stop=(q == 3))

            # ---------------- per-batch scalars to partitions --------------
            PR = pool.tile([1, 128], F32)
            V.memset(PR[:, :], 0.0)
            V.tensor_copy(PR[:, 0:8], MB[0:1, 0:8])
            V.tensor_copy(PR[:, 32:40], MB[0:1, 8:16])
            V.tensor_copy(PR[:, 64:72], sRow[:, :])
            psSC = psp.tile([128, 1], F32)
            T.transpose(psSC[:, :], PR[:, :], IDN_t[0:1, 0:1])
            SC = pool.tile([128, 1], F32)
            A.activation(SC[:, :], psSC[:, :], ACTF.Copy)
            SCC = pool.tile([8, 4], F32)
            V.tensor_copy(SCC[:, 0:1], SC[0:8, :])      # mx
            V.tensor_copy(SCC[:, 1:2], SC[32:40, :])    # my
            V.tensor_copy(SCC[:, 2:3], SC[64:72, :])    # s_dst
            V.tensor_scalar(out=SCC[:, 3:4], in0=psA[:, 72:73],
                            scalar1=NPTS * 1e-4, op0=ALU.is_gt, scalar2=None)
            IG = pool.tile([8, 1], F32)
            V.tensor_scalar(out=IG[:, :], in0=SCC[:, 3:4], scalar1=-1.0,
                            op0=ALU.mult, scalar2=1.0, op1=ALU.add)

            # ---------------- Horner / Neumann solve ----------------
            GT = pool.tile([8, 64], F32)
            V.tensor_tensor(
                out=GT[:, :].rearrange("p (i j) -> p i j", i=8, j=8),
                in0=IME_t[:, :].rearrange("p (i j) -> p i j", i=8, j=8),
                in1=psA[:, 0:72].rearrange("p (i k) -> p i k", i=8, k=9)[:, :, 0:8],
                op=ALU.subtract)
            C0 = pool.tile([8, 8], F32)
            A.activation(C0[:, :], psA[:, 8:72:9], ACTF.Copy)
            Gv3 = GT[:, :].rearrange("p (i j) -> p i j", i=8, j=8)
            PH = pool.tile([8, 64], F32)
            PHv = PH[:, :].rearrange("p (i j) -> p i j", i=8, j=8)
            YT = pool.tile([8, 8], F32)
            XC = pool.tile([8, 8], F32)
            for it in range(KHORNER):
                xin = C0 if it == 0 else XC
                V.tensor_tensor(out=PHv, in0=Gv3,
                                in1=xin[:, :].unsqueeze(1).broadcast_to([8, 8, 8]),
                                op=ALU.mult)
                V.tensor_reduce(out=YT[:, :], in_=PHv,
                                axis=mybir.AxisListType.X, op=ALU.add)
                xout = HN[:, 0:8] if it == KHORNER - 1 else XC[:, :]
                V.tensor_tensor(out=xout, in0=YT[:, :], in1=C0[:, :], op=ALU.add)

            # ---------------- denormalize + gate ----------------
            mx_sc, my_sc = SCC[:, 0:1], SCC[:, 1:2]
            s_sc, g_sc = SCC[:, 2:3], SCC[:, 3:4]
            H2 = pool.tile([8, 9], F32)
            # H2[2,2] = c_ts*h6 + d_ts*h7 + 1 (early, feeds the sign chain)
            W1 = pool.tile([8, 1], F32)
            V.tensor_scalar(out=W1[:, :], in0=HN[:, 6:7], scalar1=cc.c_ts,
                            op0=ALU.mult, scalar2=1.0, op1=ALU.add)
            V.scalar_tensor_tensor(out=H2[:, 8:9], in0=HN[:, 7:8], scalar=cc.d_ts,
                                   in1=W1[:, :], op0=ALU.mult, op1=ALU.add)
            ABSD = pool.tile([8, 1], F32)
            A.activation(ABSD[:, :], H2[:, 8:9], ACTF.Abs)
            SGN = pool.tile([8, 1], F32)
            V.tensor_scalar(out=SGN[:, :], in0=H2[:, 8:9], scalar1=0.0,
                            op0=ALU.is_lt, scalar2=-2.0, op1=ALU.mult)
            V.tensor_scalar(out=SGN[:, :], in0=SGN[:, :], scalar1=1.0,
                            op0=ALU.add, scalar2=None)
            DEN = pool.tile([8, 1], F32)
            V.tensor_scalar(out=DEN[:, :], in0=ABSD[:, :], scalar1=1e-8,
                            op0=ALU.max, scalar2=SGN[:, :], op1=ALU.mult)
            RECD = pool.tile([8, 1], F32)
            V.reciprocal(RECD[:, :], DEN[:, :])
            RG = pool.tile([8, 1], F32)
            V.tensor_tensor(out=RG[:, :], in0=RECD[:, :], in1=g_sc, op=ALU.mult)
            # rows of inv(T_dst) @ Hn
            T1 = pool.tile([8, 6], F32)
            H1 = pool.tile([8, 9], F32)
            V.tensor_scalar(out=T1[:, :], in0=HN[:, 0:6], scalar1=s_sc,
                            op0=ALU.mult, scalar2=None)
            V.scalar_tensor_tensor(out=H1[:, 0:3], in0=HN[:, 6:9], scalar=mx_sc,
                                   in1=T1[:, 0:3], op0=ALU.mult, op1=ALU.add)
            V.scalar_tensor_tensor(out=H1[:, 3:6], in0=HN[:, 6:9], scalar=my_sc,
                                   in1=T1[:, 3:6], op0=ALU.mult, op1=ALU.add)
            V.tensor_copy(H1[:, 6:9], HN[:, 6:9])
            # columns: @ T_src
            H1v = H1[:, :].rearrange("p (r c) -> p r c", r=3, c=3)
            H2v = H2[:, :].rearrange("p (r c) -> p r c", r=3, c=3)
            V.tensor_scalar(out=H2v[:, :, 0:2], in0=H1v[:, :, 0:2],
                            scalar1=cc.a_ts, op0=ALU.mult, scalar2=None)
            T2 = pool.tile([8, 2], F32)
            T3 = pool.tile([8, 2], F32)
            V.tensor_scalar(out=T2[:, :], in0=H1[:, 0:4:3], scalar1=cc.c_ts,
                            op0=ALU.mult, scalar2=None)
            V.scalar_tensor_tensor(out=T3[:, :], in0=H1[:, 1:5:3], scalar=cc.d_ts,
                                   in1=T2[:, :], op0=ALU.mult, op1=ALU.add)
            V.tensor_tensor(out=H2[:, 2:6:3], in0=T3[:, :], in1=H1[:, 2:6:3],
                            op=ALU.add)
            TI = pool.tile([8, 9], F32)
            OUTt = pool.tile([8, 9], F32)
            V.tensor_scalar(out=TI[:, :], in0=IEYE[:, :], scalar1=IG[:, :],
                            op0=ALU.mult, scalar2=None)
            V.scalar_tensor_tensor(out=OUTt[:, :], in0=H2[:, :], scalar=RG[:, :],
                                   in1=TI[:, :], op0=ALU.mult, op1=ALU.add)
            A.dma_start(Hout.ap().rearrange("b r c -> b (r c)"), OUTt[:, :])

    nc.compile()
    return nc


# ---------------------------------------------------------------------------
# host wrapper
# ---------------------------------------------------------------------------

_CACHE = {}


def _get(img_h, img_w):
    key = (int(img_h), int(img_w))
    if key not in _CACHE:
        cc = _Consts(*key)
        _CACHE[key] = (cc, _build_program(cc))
    return _CACHE[key]


def _in_maps(cc, flow, mask):
    flow = np.ascontiguousarray(flow, np.float32)
    mask = np.ascontiguousarray(mask, np.float32)
    maps = []
    for c in range(NCORES):
        fmc = np.concatenate(
            [flow[c * BPC:(c + 1) * BPC], mask[c * BPC:(c + 1) * BPC]], axis=1)
        maps.append({
            "fm": np.ascontiguousarray(fmc),
            "CMAIN": cc.CMAIN, "CW4M": cc.CW4M, "CSMALL": cc.CSMALL,
            "XIDX": cc.XIDX,
        })
    return maps


def run(flow, mask, img_h, img_w, trace=False, **spmd_kwargs):
    cc, nc = _get(img_h, img_w)
    res = bass_utils.run_bass_kernel_spmd(
        nc, _in_maps(cc, flow, mask), list(range(NCORES)), trace=trace, **spmd_kwargs
    )
    out = np.concatenate([res.results[c]["H"] for c in range(NCORES)], axis=0)
    return out.astype(np.float32), res


def kernel(flow, mask, img_h, img_w):
    out, _ = run(flow, mask, img_h, img_w)
    return out
